# revision 11
# baseline (speedup 1.0000x reference)
"""ClusterNorm1d kernel for Trainium2 (Bass/Tile), 8-core data parallel.

out[b,d,k] = sum_e Std_inv[k,d,e] * (x[b,e,k] - mu[e,k])

Strategy (v3, fp8 correction output):
  - Decompose Std_inv = I + R (R ~ 0.01, small). The device computes only
    the *correction* c = 8*[R @ (x - mu) - mu] and the host forms
    out = x + c/8 with its exact f32 copy of x. The identity term never
    moves through the device, so the output stream shrinks to fp8
    (e3m4, 4 mantissa bits; |c| <= ~6 < 15.5 max normal) and the weight
    panels shrink to fp8 (e4m3; values 8*R ~ 0.08..0.4 are all normal).
    Measured end-to-end rel err ~4e-3 against the f32 reference
    (budget 2e-2).
  - Shard batch B=8192 across 8 cores (1024 rows each).
  - Host packs x pre-transposed and pair-interleaved in fp8e4 (x only
    feeds the R-term; the identity term comes from the host's f32 x, so
    fp8 input error contributes ~1e-3):
      xt[c, j, b] = x[b, e, j + 64*p]   with c = 2e + p
    so clusters (j, j+64) share one 128-deep contraction. Weight panels
    are block-diagonal pair panels of 8R in fp8e4:
      W[c=2e+pc, j, n=2d+pd] = 8*R[j+64*pd, d, e] * (pc == pd)
  - Device work per pair j: one stationary-weight matmul (lhsT fp8e4,
    rhs fp8e4 -> psum f32) per 512-row segment, then a PSUM->SBUF drain
    fused with the bias nb[n,j] = -8*(R@mu + mu) (per-partition scalar),
    alternating ACT / DVE, writing fp8e3.
  - DMA: x stream (8.4 MB fp8) on the SP HWDGE queue; W (1 MB) + out
    stream (8.4 MB fp8) on the ACT HWDGE queue. Per-core traffic ~26 MB
    vs ~36 MB for the bf16-everything variant; fabric ceiling is
    ~430 GB/s.
"""

import numpy as np

B, D, K = 8192, 64, 128
N_CORES = 8
B_SHARD = B // N_CORES  # 1024
P = 128                 # SBUF partitions
NPAIR = K // 2          # 64 cluster pairs: (j, j+64)
SEG = 512               # matmul moving free-dim per PSUM bank
CSCALE = 8.0            # correction scale: device returns 8*c, host divides

# DMA chunking (pairs per transfer). Packet size per partition row is
# chunk*b_shard*dtype bytes. Head chunks small so compute starts early,
# tail chunks small so the pipeline drains early.
IN_CHUNKS = [8, 16, 16, 16, 4, 2, 1, 1]
OUT_CHUNKS = [8] * 7 + [4, 2, 1, 1]

_cache = {}


def _bounds(chunks):
    out, s = [], 0
    for c in chunks:
        out.append((s, c))
        s += c
    return out


def _build_nc(b_shard):
    import concourse.tile as tile
    from concourse import bacc, mybir

    f32 = mybir.dt.float32
    bf16 = mybir.dt.bfloat16
    fp8e4 = mybir.dt.float8e4
    fp8e3 = mybir.dt.float8e3
    nc = bacc.Bacc("TRN2", target_bir_lowering=False)

    xt_d = nc.dram_tensor("xt", [P, NPAIR, b_shard], fp8e4, kind="ExternalInput")
    w_d = nc.dram_tensor("w", [P, NPAIR, P], fp8e4, kind="ExternalInput")
    nb_d = nc.dram_tensor("nbias", [P, NPAIR], f32, kind="ExternalInput")
    o_d = nc.dram_tensor("out", [P, NPAIR, b_shard], fp8e3, kind="ExternalOutput")

    seg = min(SEG, b_shard)
    nseg = b_shard // seg
    in_bounds = _bounds(IN_CHUNKS)
    out_bounds = _bounds(OUT_CHUNKS)

    with tile.TileContext(nc) as tc:
        with (
            tc.tile_pool(name="consts", bufs=1) as consts,
            tc.tile_pool(name="xin", bufs=4) as xin,
            tc.tile_pool(name="oout", bufs=5) as oout,
            tc.tile_pool(name="ps", bufs=4, space="PSUM") as psp,
        ):
            w_sb = consts.tile([P, NPAIR, P], fp8e4)
            nb_sb = consts.tile([P, NPAIR], f32)

            # Engine warm-ups on a zeroed local tile — independent of the
            # const DMAs so they run during the queue spin-up.
            warm_in = consts.tile([P, P], bf16)
            nc.gpsimd.memset(warm_in, 0)
            scratch = consts.tile([P, 2], f32)
            nc.gpsimd.memset(scratch, 0)
            warm_ps = psp.tile([P, 2, seg], f32, tag="ps")
            nc.tensor.matmul(warm_ps[:, 0, 0:P], lhsT=warm_in, rhs=warm_in)
            nc.scalar.copy(out=scratch[:, 0:1], in_=scratch[:, 0:1])
            nc.vector.tensor_copy(out=scratch[:, 1:2], in_=scratch[:, 1:2])

            # HWDGE descriptor generation is ~24ns/descriptor (~3.1us per
            # 128-row DMA entry) and serializes per queue. The SP ring leads
            # with the first x chunks; the fp8 W panel rides the ACT ring
            # ahead of the out stream, split so the first pairs' weights
            # land ~4us earlier than a single 1 MB entry would.
            nc.scalar.dma_start(out=w_sb[:, 0:8, :], in_=w_d[:, 0:8, :])
            nc.scalar.dma_start(out=nb_sb, in_=nb_d[:])
            nc.scalar.dma_start(out=w_sb[:, 8:24, :], in_=w_d[:, 8:24, :])
            nc.scalar.dma_start(out=w_sb[:, 24:NPAIR, :],
                                in_=w_d[:, 24:NPAIR, :])

            in_it = iter(in_bounds)
            out_it = iter(out_bounds)
            xt = o_sb = None
            in_s = in_n = out_s = out_n = 0
            for j in range(NPAIR):
                if xt is None or j >= in_s + in_n:
                    in_s, in_n = next(in_it)
                    xt = xin.tile([P, in_n, b_shard], fp8e4, tag="xt")
                    nc.sync.dma_start(
                        out=xt, in_=xt_d[:, in_s:in_s + in_n, :])
                if o_sb is None or j >= out_s + out_n:
                    out_s, out_n = next(out_it)
                    o_sb = oout.tile([P, out_n, b_shard], fp8e3, tag="o")
                # both halves of pair j land in one 2-bank PSUM tile, then
                # drain in a single bias-fused op (alternating ACT/DVE)
                ps = psp.tile([P, nseg, seg], f32, tag="ps")
                for h in range(nseg):
                    nc.tensor.matmul(
                        ps[:, h, :], lhsT=w_sb[:, j, :],
                        rhs=xt[:, j - in_s, h * seg:(h + 1) * seg])
                dst = o_sb[:, j - out_s, :]
                src = ps.rearrange("p a b -> p (a b)")
                nbj = nb_sb[:, j:j + 1]
                if j % 2 == 0:
                    nc.scalar.add(dst, src, nbj)
                else:
                    nc.vector.tensor_scalar_add(dst, src, nbj)
                if j == out_s + out_n - 1:
                    nc.scalar.dma_start(
                        out=o_d[:, out_s:out_s + out_n, :], in_=o_sb)

    nc.compile()
    return nc


def _host_prep(mu_track, Std_inv_track):
    """Block-diagonal pair panels of the residual W[c=2e+pc, j, n=2d+pd] =
    8*R[j+64pd, d, e]*(pc==pd) in fp8e4, and the negated per-partition bias
    nbias[n=2d+p, j] = -8*(R@mu + mu)[d, j+64p] (f32), where R = S - I."""
    import ml_dtypes

    S = np.ascontiguousarray(Std_inv_track, dtype=np.float32)
    mu = np.ascontiguousarray(mu_track, dtype=np.float32)
    R = S - np.eye(D, dtype=np.float32)[None]

    W = np.zeros((2 * D, NPAIR, 2 * D), dtype=np.float32)
    W6 = W.reshape(D, 2, NPAIR, D, 2)                 # [e, pc, j, d, pd]
    R_r = (CSCALE * R).reshape(2, NPAIR, D, D)        # [pk, j, d, e]
    W6[:, 0, :, :, 0] = R_r[0].transpose(2, 0, 1)     # [e, j, d]
    W6[:, 1, :, :, 1] = R_r[1].transpose(2, 0, 1)

    bias_dk = np.einsum("kde,ek->dk", R, mu) + mu     # [d, k], k = 64p + j
    nbias = (-CSCALE) * bias_dk.reshape(D, 2, NPAIR).reshape(2 * D, NPAIR)
    return (W.astype(ml_dtypes.float8_e4m3),
            np.ascontiguousarray(nbias, dtype=np.float32))


def _pack_x(x, n_cores, b_shard):
    """x [n_cores*b_shard, D, K] f32 -> xt [n_cores, 128, NPAIR, b_shard] bf16
    with xt[core, 2e+p, j, b] = x[b, e, j + 64p]."""
    import ml_dtypes

    xb = np.ascontiguousarray(x, dtype=np.float32).astype(ml_dtypes.float8_e4m3)
    xp = xb.reshape(n_cores, b_shard, D, 2, NPAIR)    # [core, b, e, p, j]
    xt = xp.transpose(0, 2, 3, 4, 1)                  # [core, e, p, j, b]
    return np.ascontiguousarray(xt).reshape(n_cores, P, NPAIR, b_shard)


def _unpack_out(oT, x, n_cores, b_shard):
    """oT [n_cores, 128, NPAIR, b_shard] fp8e3 (the scaled correction 8c with
    c[b, d, j+64p] = oT[core, 2d+p, j, b]) -> out = x + c/8 [B, D, K] f32."""
    ov = np.asarray(oT).astype(np.float32)
    ov = ov.reshape(n_cores, D, 2, NPAIR, b_shard)    # [core, d, p, j, b]
    c = ov.transpose(0, 4, 1, 2, 3)                   # [core, b, d, p, j]
    c = np.ascontiguousarray(c).reshape(n_cores * b_shard, D, K)
    return np.asarray(x, dtype=np.float32) + c * (1.0 / CSCALE)


def kernel(x, mu_track, Std_inv_track):
    from concourse.bass_utils import run_bass_kernel_spmd

    xt = _pack_x(x, N_CORES, B_SHARD)
    W, nbias = _host_prep(mu_track, Std_inv_track)

    if "nc" not in _cache:
        _cache["nc"] = _build_nc(B_SHARD)
    nc = _cache["nc"]

    in_maps = []
    for i in range(N_CORES):
        in_maps.append({"xt": xt[i], "w": W, "nbias": nbias})
    res = run_bass_kernel_spmd(nc, in_maps, core_ids=list(range(N_CORES)))
    oT = np.stack([r["out"] for r in res.results], axis=0)
    return _unpack_out(oT, x, N_CORES, B_SHARD)


# revision 12
# speedup vs baseline: 1.1505x; 1.1505x over previous
"""ClusterNorm1d kernel for Trainium2 (Bass/Tile), 8-core data parallel.

out[b,d,k] = sum_e Std_inv[k,d,e] * (x[b,e,k] - mu[e,k])

Strategy (v3, fp8 correction output):
  - Decompose Std_inv = I + R (R ~ 0.01, small). The device computes only
    the *correction* c = 8*[R @ (x - mu) - mu] and the host forms
    out = x + c/8 with its exact f32 copy of x. The identity term never
    moves through the device, so the output stream shrinks to fp8
    (e3m4, 4 mantissa bits; |c| <= ~6 < 15.5 max normal) and the weight
    panels shrink to fp8 (e4m3; values 8*R ~ 0.08..0.4 are all normal).
    Measured end-to-end rel err ~4e-3 against the f32 reference
    (budget 2e-2).
  - Shard batch B=8192 across 8 cores (1024 rows each).
  - Host packs x pre-transposed and pair-interleaved in fp8e4 (x only
    feeds the R-term; the identity term comes from the host's f32 x, so
    fp8 input error contributes ~1e-3):
      xt[c, j, b] = x[b, e, j + 64*p]   with c = 2e + p
    so clusters (j, j+64) share one 128-deep contraction. Weight panels
    are block-diagonal pair panels of 8R in fp8e4:
      W[c=2e+pc, j, n=2d+pd] = 8*R[j+64*pd, d, e] * (pc == pd)
  - Device work per pair j: one stationary-weight matmul (lhsT fp8e4,
    rhs fp8e4 -> psum f32) per 512-row segment, then a PSUM->SBUF drain
    fused with the bias nb[n,j] = -8*(R@mu + mu) (per-partition scalar),
    alternating ACT / DVE, writing fp8e3.
  - DMA: x stream (8.4 MB fp8) on the SP HWDGE queue; W (1 MB) + out
    stream (8.4 MB fp8) on the ACT HWDGE queue. Per-core traffic ~26 MB
    vs ~36 MB for the bf16-everything variant; fabric ceiling is
    ~430 GB/s.
"""

import numpy as np

B, D, K = 8192, 64, 128
N_CORES = 8
B_SHARD = B // N_CORES  # 1024
P = 128                 # SBUF partitions
NPAIR = K // 2          # 64 cluster pairs: (j, j+64)
SEG = 512               # matmul moving free-dim per PSUM bank
CSCALE = 8.0            # correction scale: device returns 8*c, host divides

# DMA chunking (pairs per transfer). Packet size per partition row is
# chunk*b_shard*dtype bytes. Head chunks small so compute starts early,
# tail chunks small so the pipeline drains early.
IN_CHUNKS = [4, 8, 16, 16, 16, 2, 1, 1]
OUT_CHUNKS = [8] * 7 + [4, 2, 1, 1]

_cache = {}


def _bounds(chunks):
    out, s = [], 0
    for c in chunks:
        out.append((s, c))
        s += c
    return out


def _build_nc(b_shard):
    import concourse.tile as tile
    from concourse import bacc, mybir

    f32 = mybir.dt.float32
    bf16 = mybir.dt.bfloat16
    fp8e4 = mybir.dt.float8e4
    fp8e3 = mybir.dt.float8e3
    nc = bacc.Bacc("TRN2", target_bir_lowering=False)

    xt_d = nc.dram_tensor("xt", [P, NPAIR, b_shard], fp8e4, kind="ExternalInput")
    w_d = nc.dram_tensor("w", [P, NPAIR, P], fp8e4, kind="ExternalInput")
    nb_d = nc.dram_tensor("nbias", [P, NPAIR], f32, kind="ExternalInput")
    o_d = nc.dram_tensor("out", [P, NPAIR, b_shard], fp8e3, kind="ExternalOutput")

    seg = min(SEG, b_shard)
    nseg = b_shard // seg
    in_bounds = _bounds(IN_CHUNKS)
    out_bounds = _bounds(OUT_CHUNKS)

    with tile.TileContext(nc) as tc:
        with (
            tc.tile_pool(name="consts", bufs=1) as consts,
            tc.tile_pool(name="xin", bufs=4) as xin,
            tc.tile_pool(name="oout", bufs=5) as oout,
            tc.tile_pool(name="ps", bufs=4, space="PSUM") as psp,
        ):
            w_sb = consts.tile([P, NPAIR, P], fp8e4)
            nb_sb = consts.tile([P, NPAIR], f32)

            # Engine warm-ups on a zeroed local tile — independent of the
            # const DMAs so they run during the queue spin-up.
            warm_in = consts.tile([P, P], bf16)
            nc.gpsimd.memset(warm_in, 0)
            scratch = consts.tile([P, 2], f32)
            nc.gpsimd.memset(scratch, 0)
            warm_ps = psp.tile([P, 2, seg], f32, tag="ps")
            nc.tensor.matmul(warm_ps[:, 0, 0:P], lhsT=warm_in, rhs=warm_in)
            nc.scalar.copy(out=scratch[:, 0:1], in_=scratch[:, 0:1])
            nc.vector.tensor_copy(out=scratch[:, 1:2], in_=scratch[:, 1:2])

            # HWDGE descriptor generation is ~24ns/descriptor (~3.1us per
            # 128-row DMA entry) and serializes per queue. The SP ring leads
            # with the first x chunks; the fp8 W panel rides the ACT ring
            # ahead of the out stream, split so the first pairs' weights
            # land ~4us earlier than a single 1 MB entry would.
            nc.scalar.dma_start(out=w_sb[:, 0:8, :], in_=w_d[:, 0:8, :])
            nc.scalar.dma_start(out=w_sb[:, 8:NPAIR, :],
                                in_=w_d[:, 8:NPAIR, :])

            in_it = iter(in_bounds)
            out_it = iter(out_bounds)
            xt = o_sb = None
            in_s = in_n = out_s = out_n = 0
            for j in range(NPAIR):
                if xt is None or j >= in_s + in_n:
                    in_s, in_n = next(in_it)
                    xt = xin.tile([P, in_n, b_shard], fp8e4, tag="xt")
                    nc.sync.dma_start(
                        out=xt, in_=xt_d[:, in_s:in_s + in_n, :])
                    if j == 0:
                        nc.sync.dma_start(out=nb_sb, in_=nb_d[:])
                if o_sb is None or j >= out_s + out_n:
                    out_s, out_n = next(out_it)
                    o_sb = oout.tile([P, out_n, b_shard], fp8e3, tag="o")
                # both halves of pair j land in one 2-bank PSUM tile, then
                # drain in a single bias-fused op (alternating ACT/DVE)
                ps = psp.tile([P, nseg, seg], f32, tag="ps")
                for h in range(nseg):
                    nc.tensor.matmul(
                        ps[:, h, :], lhsT=w_sb[:, j, :],
                        rhs=xt[:, j - in_s, h * seg:(h + 1) * seg])
                dst = o_sb[:, j - out_s, :]
                src = ps.rearrange("p a b -> p (a b)")
                nbj = nb_sb[:, j:j + 1]
                if j % 2 == 0:
                    nc.scalar.add(dst, src, nbj)
                else:
                    nc.vector.tensor_scalar_add(dst, src, nbj)
                if j == out_s + out_n - 1:
                    nc.scalar.dma_start(
                        out=o_d[:, out_s:out_s + out_n, :], in_=o_sb)

    nc.compile()
    return nc


def _host_prep(mu_track, Std_inv_track):
    """Block-diagonal pair panels of the residual W[c=2e+pc, j, n=2d+pd] =
    8*R[j+64pd, d, e]*(pc==pd) in fp8e4, and the negated per-partition bias
    nbias[n=2d+p, j] = -8*(R@mu + mu)[d, j+64p] (f32), where R = S - I."""
    import ml_dtypes

    S = np.ascontiguousarray(Std_inv_track, dtype=np.float32)
    mu = np.ascontiguousarray(mu_track, dtype=np.float32)
    R = S - np.eye(D, dtype=np.float32)[None]

    W = np.zeros((2 * D, NPAIR, 2 * D), dtype=np.float32)
    W6 = W.reshape(D, 2, NPAIR, D, 2)                 # [e, pc, j, d, pd]
    R_r = (CSCALE * R).reshape(2, NPAIR, D, D)        # [pk, j, d, e]
    W6[:, 0, :, :, 0] = R_r[0].transpose(2, 0, 1)     # [e, j, d]
    W6[:, 1, :, :, 1] = R_r[1].transpose(2, 0, 1)

    bias_dk = np.einsum("kde,ek->dk", R, mu) + mu     # [d, k], k = 64p + j
    nbias = (-CSCALE) * bias_dk.reshape(D, 2, NPAIR).reshape(2 * D, NPAIR)
    return (W.astype(ml_dtypes.float8_e4m3),
            np.ascontiguousarray(nbias, dtype=np.float32))


def _pack_x(x, n_cores, b_shard):
    """x [n_cores*b_shard, D, K] f32 -> xt [n_cores, 128, NPAIR, b_shard] bf16
    with xt[core, 2e+p, j, b] = x[b, e, j + 64p]."""
    import ml_dtypes

    xb = np.ascontiguousarray(x, dtype=np.float32).astype(ml_dtypes.float8_e4m3)
    xp = xb.reshape(n_cores, b_shard, D, 2, NPAIR)    # [core, b, e, p, j]
    xt = xp.transpose(0, 2, 3, 4, 1)                  # [core, e, p, j, b]
    return np.ascontiguousarray(xt).reshape(n_cores, P, NPAIR, b_shard)


def _unpack_out(oT, x, n_cores, b_shard):
    """oT [n_cores, 128, NPAIR, b_shard] fp8e3 (the scaled correction 8c with
    c[b, d, j+64p] = oT[core, 2d+p, j, b]) -> out = x + c/8 [B, D, K] f32."""
    ov = np.asarray(oT).astype(np.float32)
    ov = ov.reshape(n_cores, D, 2, NPAIR, b_shard)    # [core, d, p, j, b]
    c = ov.transpose(0, 4, 1, 2, 3)                   # [core, b, d, p, j]
    c = np.ascontiguousarray(c).reshape(n_cores * b_shard, D, K)
    return np.asarray(x, dtype=np.float32) + c * (1.0 / CSCALE)


def kernel(x, mu_track, Std_inv_track):
    from concourse.bass_utils import run_bass_kernel_spmd

    xt = _pack_x(x, N_CORES, B_SHARD)
    W, nbias = _host_prep(mu_track, Std_inv_track)

    if "nc" not in _cache:
        _cache["nc"] = _build_nc(B_SHARD)
    nc = _cache["nc"]

    in_maps = []
    for i in range(N_CORES):
        in_maps.append({"xt": xt[i], "w": W, "nbias": nbias})
    res = run_bass_kernel_spmd(nc, in_maps, core_ids=list(range(N_CORES)))
    oT = np.stack([r["out"] for r in res.results], axis=0)
    return _unpack_out(oT, x, N_CORES, B_SHARD)


# revision 14
# speedup vs baseline: 1.2298x; 1.0689x over previous
"""ClusterNorm1d kernel for Trainium2 (Bass/Tile), 8-core data parallel.

out[b,d,k] = sum_e Std_inv[k,d,e] * (x[b,e,k] - mu[e,k])

Strategy (v8, fp8 correction output):
  - Decompose Std_inv = I + R (R ~ 0.01, small). The device computes only
    the *correction* c = 8*[R @ (x - mu) - mu] and the host forms
    out = x + c/8 with its exact f32 copy of x. The identity term never
    moves through the device, so the output stream shrinks to fp8
    (e3m4, 4 mantissa bits; |c| <= ~6 < 15.5 max normal) and the weight
    panels shrink to fp8 (e4m3; values 8*R ~ 0.08..0.4 are all normal).
    Measured end-to-end rel err ~4e-3 against the f32 reference
    (budget 2e-2).
  - Shard batch B=8192 across 8 cores (1024 rows each).
  - Host packs x pre-transposed and pair-interleaved in fp8e4 (x only
    feeds the R-term; the identity term comes from the host's f32 x, so
    fp8 input error contributes ~1e-3):
      xt[c, j, b] = x[b, e, j + 64*p]   with c = 2e + p
    so clusters (j, j+64) share one 128-deep contraction. Weight panels
    are block-diagonal pair panels of 8R in fp8e4:
      W[c=2e+pc, j, n=2d+pd] = 8*R[j+64*pd, d, e] * (pc == pd)
  - Device work per pair j: one stationary-weight matmul (lhsT fp8e4,
    rhs fp8e4 -> psum f32) per 512-row segment, then a PSUM->SBUF drain
    fused with the bias nb[n,j] = -8*(R@mu + mu) (per-partition scalar),
    alternating ACT / DVE, writing fp8e3.
  - DMA: x stream (8.4 MB fp8) + nbias on the SP HWDGE queue; W (1 MB,
    split [8, 56] so the first pairs' weights land ~4us early) + out
    stream (8.4 MB fp8) on the ACT HWDGE queue. ~17.9 MB/core total vs
    ~36 MB for the bf16-everything variant; fabric ceiling ~430 GB/s.
    With DMA off the critical path the kernel is PE-stream-bound:
    64 pairs x 2 x 512-row fp8 matmuls at ~0.74 ns/row (~49 us) plus
    per-pair LDWEIGHTS, ~95% dense once the W head chunk lands.
    (fp8 DoubleRow was tried and is NOT faster on this hardware; it
    runs at ~1 row/cycle in both plain and interleaved layouts.)
"""

import numpy as np

B, D, K = 8192, 64, 128
N_CORES = 8
B_SHARD = B // N_CORES  # 1024
P = 128                 # SBUF partitions
NPAIR = K // 2          # 64 cluster pairs: (j, j+64)
SEG = 512               # matmul moving free-dim per PSUM bank
CSCALE = 8.0            # correction scale: device returns 8*c, host divides

# DMA chunking (pairs per transfer). Packet size per partition row is
# chunk*b_shard*dtype bytes. Head chunks small so compute starts early,
# tail chunks small so the pipeline drains early.
IN_CHUNKS = [4, 8, 16, 16, 16, 2, 1, 1]
OUT_CHUNKS = [16, 16, 16, 8, 4, 2, 1, 1]

_cache = {}


def _bounds(chunks):
    out, s = [], 0
    for c in chunks:
        out.append((s, c))
        s += c
    return out


def _build_nc(b_shard):
    import concourse.tile as tile
    from concourse import bacc, mybir

    f32 = mybir.dt.float32
    bf16 = mybir.dt.bfloat16
    fp8e4 = mybir.dt.float8e4
    fp8e3 = mybir.dt.float8e3
    nc = bacc.Bacc("TRN2", target_bir_lowering=False)

    xt_d = nc.dram_tensor("xt", [P, NPAIR, b_shard], fp8e4, kind="ExternalInput")
    w_d = nc.dram_tensor("w", [P, NPAIR, P], fp8e4, kind="ExternalInput")
    nb_d = nc.dram_tensor("nbias", [P, NPAIR], f32, kind="ExternalInput")
    o_d = nc.dram_tensor("out", [P, NPAIR, b_shard], fp8e3, kind="ExternalOutput")

    seg = min(SEG, b_shard)
    nseg = b_shard // seg
    in_bounds = _bounds(IN_CHUNKS)
    out_bounds = _bounds(OUT_CHUNKS)

    with tile.TileContext(nc) as tc:
        with (
            tc.tile_pool(name="consts", bufs=1) as consts,
            tc.tile_pool(name="xin", bufs=4) as xin,
            tc.tile_pool(name="oout", bufs=3) as oout,
            tc.tile_pool(name="ps", bufs=4, space="PSUM") as psp,
        ):
            w_sb = consts.tile([P, NPAIR, P], fp8e4)
            nb_sb = consts.tile([P, NPAIR], f32)

            # Engine warm-ups on a zeroed local tile — independent of the
            # const DMAs so they run during the queue spin-up.
            warm_in = consts.tile([P, P], bf16)
            nc.gpsimd.memset(warm_in, 0)
            scratch = consts.tile([P, 2], f32)
            nc.gpsimd.memset(scratch, 0)
            warm_ps = psp.tile([P, 2, seg], f32, tag="ps")
            nc.tensor.matmul(warm_ps[:, 0, 0:P], lhsT=warm_in, rhs=warm_in)
            nc.scalar.copy(out=scratch[:, 0:1], in_=scratch[:, 0:1])
            nc.vector.tensor_copy(out=scratch[:, 1:2], in_=scratch[:, 1:2])

            # HWDGE descriptor generation is ~24ns/descriptor (~3.1us per
            # 128-row DMA entry) and serializes per queue. The SP ring leads
            # with the first x chunks; the fp8 W panel rides the ACT ring
            # ahead of the out stream, split so the first pairs' weights
            # land ~4us earlier than a single 1 MB entry would.
            nc.scalar.dma_start(out=w_sb[:, 0:8, :], in_=w_d[:, 0:8, :])
            nc.scalar.dma_start(out=w_sb[:, 8:NPAIR, :],
                                in_=w_d[:, 8:NPAIR, :])

            in_it = iter(in_bounds)
            out_it = iter(out_bounds)
            xt = o_sb = None
            in_s = in_n = out_s = out_n = 0
            for j in range(NPAIR):
                if xt is None or j >= in_s + in_n:
                    in_s, in_n = next(in_it)
                    xt = xin.tile([P, in_n, b_shard], fp8e4, tag="xt")
                    nc.sync.dma_start(
                        out=xt, in_=xt_d[:, in_s:in_s + in_n, :])
                    if j == 0:
                        nc.sync.dma_start(out=nb_sb, in_=nb_d[:])
                if o_sb is None or j >= out_s + out_n:
                    out_s, out_n = next(out_it)
                    o_sb = oout.tile([P, out_n, b_shard], fp8e3, tag="o")
                # both halves of pair j land in one 2-bank PSUM tile, then
                # drain in a single bias-fused op (alternating ACT/DVE)
                ps = psp.tile([P, nseg, seg], f32, tag="ps")
                for h in range(nseg):
                    nc.tensor.matmul(
                        ps[:, h, :], lhsT=w_sb[:, j, :],
                        rhs=xt[:, j - in_s, h * seg:(h + 1) * seg])
                dst = o_sb[:, j - out_s, :]
                src = ps.rearrange("p a b -> p (a b)")
                nbj = nb_sb[:, j:j + 1]
                if j % 2 == 0:
                    nc.scalar.add(dst, src, nbj)
                else:
                    nc.vector.tensor_scalar_add(dst, src, nbj)
                if j == out_s + out_n - 1:
                    nc.sync.dma_start(
                        out=o_d[:, out_s:out_s + out_n, :], in_=o_sb)

    nc.compile()
    return nc


def _host_prep(mu_track, Std_inv_track):
    """Block-diagonal pair panels of the residual W[c=2e+pc, j, n=2d+pd] =
    8*R[j+64pd, d, e]*(pc==pd) in fp8e4, and the negated per-partition bias
    nbias[n=2d+p, j] = -8*(R@mu + mu)[d, j+64p] (f32), where R = S - I."""
    import ml_dtypes

    S = np.ascontiguousarray(Std_inv_track, dtype=np.float32)
    mu = np.ascontiguousarray(mu_track, dtype=np.float32)
    R = S - np.eye(D, dtype=np.float32)[None]

    W = np.zeros((2 * D, NPAIR, 2 * D), dtype=np.float32)
    W6 = W.reshape(D, 2, NPAIR, D, 2)                 # [e, pc, j, d, pd]
    R_r = (CSCALE * R).reshape(2, NPAIR, D, D)        # [pk, j, d, e]
    W6[:, 0, :, :, 0] = R_r[0].transpose(2, 0, 1)     # [e, j, d]
    W6[:, 1, :, :, 1] = R_r[1].transpose(2, 0, 1)

    bias_dk = np.einsum("kde,ek->dk", R, mu) + mu     # [d, k], k = 64p + j
    nbias = (-CSCALE) * bias_dk.reshape(D, 2, NPAIR).reshape(2 * D, NPAIR)
    return (W.astype(ml_dtypes.float8_e4m3),
            np.ascontiguousarray(nbias, dtype=np.float32))


def _pack_x(x, n_cores, b_shard):
    """x [n_cores*b_shard, D, K] f32 -> xt [n_cores, 128, NPAIR, b_shard] bf16
    with xt[core, 2e+p, j, b] = x[b, e, j + 64p]."""
    import ml_dtypes

    xb = np.ascontiguousarray(x, dtype=np.float32).astype(ml_dtypes.float8_e4m3)
    xp = xb.reshape(n_cores, b_shard, D, 2, NPAIR)    # [core, b, e, p, j]
    xt = xp.transpose(0, 2, 3, 4, 1)                  # [core, e, p, j, b]
    return np.ascontiguousarray(xt).reshape(n_cores, P, NPAIR, b_shard)


def _unpack_out(oT, x, n_cores, b_shard):
    """oT [n_cores, 128, NPAIR, b_shard] fp8e3 (the scaled correction 8c with
    c[b, d, j+64p] = oT[core, 2d+p, j, b]) -> out = x + c/8 [B, D, K] f32."""
    ov = np.asarray(oT).astype(np.float32)
    ov = ov.reshape(n_cores, D, 2, NPAIR, b_shard)    # [core, d, p, j, b]
    c = ov.transpose(0, 4, 1, 2, 3)                   # [core, b, d, p, j]
    c = np.ascontiguousarray(c).reshape(n_cores * b_shard, D, K)
    return np.asarray(x, dtype=np.float32) + c * (1.0 / CSCALE)


def kernel(x, mu_track, Std_inv_track):
    from concourse.bass_utils import run_bass_kernel_spmd

    xt = _pack_x(x, N_CORES, B_SHARD)
    W, nbias = _host_prep(mu_track, Std_inv_track)

    if "nc" not in _cache:
        _cache["nc"] = _build_nc(B_SHARD)
    nc = _cache["nc"]

    in_maps = []
    for i in range(N_CORES):
        in_maps.append({"xt": xt[i], "w": W, "nbias": nbias})
    res = run_bass_kernel_spmd(nc, in_maps, core_ids=list(range(N_CORES)))
    oT = np.stack([r["out"] for r in res.results], axis=0)
    return _unpack_out(oT, x, N_CORES, B_SHARD)


# revision 15
# speedup vs baseline: 1.2868x; 1.0463x over previous
"""ClusterNorm1d kernel for Trainium2 (Bass/Tile), 8-core data parallel.

out[b,d,k] = sum_e Std_inv[k,d,e] * (x[b,e,k] - mu[e,k])

Strategy (v8, fp8 correction output):
  - Decompose Std_inv = I + R (R ~ 0.01, small). The device computes only
    the *correction* c = 8*[R @ (x - mu) - mu] and the host forms
    out = x + c/8 with its exact f32 copy of x. The identity term never
    moves through the device, so the output stream shrinks to fp8
    (e3m4, 4 mantissa bits; |c| <= ~6 < 15.5 max normal) and the weight
    panels shrink to fp8 (e4m3; values 8*R ~ 0.08..0.4 are all normal).
    Measured end-to-end rel err ~4e-3 against the f32 reference
    (budget 2e-2).
  - Shard batch B=8192 across 8 cores (1024 rows each).
  - Host packs x pre-transposed and pair-interleaved in fp8e4 (x only
    feeds the R-term; the identity term comes from the host's f32 x, so
    fp8 input error contributes ~1e-3):
      xt[c, j, b] = x[b, e, j + 64*p]   with c = 2e + p
    so clusters (j, j+64) share one 128-deep contraction. Weight panels
    are block-diagonal pair panels of 8R in fp8e4:
      W[c=2e+pc, j, n=2d+pd] = 8*R[j+64*pd, d, e] * (pc == pd)
  - Device work per pair j: one stationary-weight matmul (lhsT fp8e4,
    rhs fp8e4 -> psum f32) per 512-row segment, then a PSUM->SBUF drain
    fused with the bias nb[n,j] = -8*(R@mu + mu) (per-partition scalar),
    alternating ACT / DVE, writing fp8e3.
  - DMA: x stream (8.4 MB fp8) + nbias on the SP HWDGE queue; W (1 MB,
    split [8, 56] so the first pairs' weights land ~4us early) + out
    stream (8.4 MB fp8) on the ACT HWDGE queue. ~17.9 MB/core total vs
    ~36 MB for the bf16-everything variant; fabric ceiling ~430 GB/s.
    With DMA off the critical path the kernel is PE-stream-bound:
    64 pairs x 2 x 512-row fp8 matmuls at ~0.74 ns/row (~49 us) plus
    per-pair LDWEIGHTS, ~95% dense once the W head chunk lands.
    (fp8 DoubleRow was tried and is NOT faster on this hardware; it
    runs at ~1 row/cycle in both plain and interleaved layouts.)
"""

import numpy as np

B, D, K = 8192, 64, 128
N_CORES = 8
B_SHARD = B // N_CORES  # 1024
P = 128                 # SBUF partitions
NPAIR = K // 2          # 64 cluster pairs: (j, j+64)
SEG = 512               # matmul moving free-dim per PSUM bank
CSCALE = 8.0            # correction scale: device returns 8*c, host divides

# DMA chunking (pairs per transfer). Packet size per partition row is
# chunk*b_shard*dtype bytes. Head chunks small so compute starts early,
# tail chunks small so the pipeline drains early.
IN_CHUNKS = [4, 8, 16, 16, 16, 2, 1, 1]
OUT_CHUNKS = [16, 16, 16, 8, 4, 2, 1, 1]

_cache = {}


def _bounds(chunks):
    out, s = [], 0
    for c in chunks:
        out.append((s, c))
        s += c
    return out


def _build_nc(b_shard):
    import concourse.tile as tile
    from concourse import bacc, mybir

    f32 = mybir.dt.float32
    bf16 = mybir.dt.bfloat16
    fp8e4 = mybir.dt.float8e4
    fp8e3 = mybir.dt.float8e3
    nc = bacc.Bacc("TRN2", target_bir_lowering=False)

    xt_d = nc.dram_tensor("xt", [P, NPAIR, b_shard], fp8e4, kind="ExternalInput")
    w_d = nc.dram_tensor("w", [P, NPAIR, P], fp8e4, kind="ExternalInput")
    nb_d = nc.dram_tensor("nbias", [P, NPAIR], f32, kind="ExternalInput")
    o_d = nc.dram_tensor("out", [P, NPAIR, b_shard], fp8e3, kind="ExternalOutput")

    seg = min(SEG, b_shard)
    nseg = b_shard // seg
    in_bounds = _bounds(IN_CHUNKS)
    out_bounds = _bounds(OUT_CHUNKS)

    with tile.TileContext(nc) as tc:
        with (
            tc.tile_pool(name="consts", bufs=1) as consts,
            tc.tile_pool(name="xin", bufs=4) as xin,
            tc.tile_pool(name="oout", bufs=3) as oout,
            tc.tile_pool(name="ps", bufs=4, space="PSUM") as psp,
        ):
            w_sb = consts.tile([P, NPAIR, P], fp8e4)
            nb_sb = consts.tile([P, NPAIR], f32)

            # Engine warm-ups on a zeroed local tile — independent of the
            # const DMAs so they run during the queue spin-up.
            warm_in = consts.tile([P, P], bf16)
            nc.gpsimd.memset(warm_in, 0)
            scratch = consts.tile([P, 2], f32)
            nc.gpsimd.memset(scratch, 0)
            warm_ps = psp.tile([P, 2, seg], f32, tag="ps")
            nc.tensor.matmul(warm_ps[:, 0, 0:P], lhsT=warm_in, rhs=warm_in)
            nc.scalar.copy(out=scratch[:, 0:1], in_=scratch[:, 0:1])
            nc.vector.tensor_copy(out=scratch[:, 1:2], in_=scratch[:, 1:2])

            # HWDGE descriptor generation is ~24ns/descriptor (~3.1us per
            # 128-row DMA entry) and serializes per queue. The SP ring leads
            # with the first x chunks; the fp8 W panel rides the ACT ring
            # ahead of the out stream, split so the first pairs' weights
            # land ~4us earlier than a single 1 MB entry would.
            nc.scalar.dma_start(out=w_sb[:, 0:8, :], in_=w_d[:, 0:8, :])
            nc.scalar.dma_start(out=w_sb[:, 8:NPAIR, :],
                                in_=w_d[:, 8:NPAIR, :])

            in_it = iter(in_bounds)
            out_it = iter(out_bounds)
            xt = o_sb = None
            in_s = in_n = out_s = out_n = 0
            for j in range(NPAIR):
                if xt is None or j >= in_s + in_n:
                    in_s, in_n = next(in_it)
                    xt = xin.tile([P, in_n, b_shard], fp8e4, tag="xt")
                    nc.sync.dma_start(
                        out=xt, in_=xt_d[:, in_s:in_s + in_n, :])
                    if j == 0:
                        nc.sync.dma_start(out=nb_sb, in_=nb_d[:])
                if o_sb is None or j >= out_s + out_n:
                    out_s, out_n = next(out_it)
                    o_sb = oout.tile([P, out_n, b_shard], fp8e3, tag="o")
                # both halves of pair j land in one 2-bank PSUM tile, then
                # drain in a single bias-fused op (alternating ACT/DVE)
                ps = psp.tile([P, nseg, seg], f32, tag="ps")
                for h in range(nseg):
                    nc.tensor.matmul(
                        ps[:, h, :], lhsT=w_sb[:, j, :],
                        rhs=xt[:, j - in_s, h * seg:(h + 1) * seg])
                dst = o_sb[:, j - out_s, :]
                src = ps.rearrange("p a b -> p (a b)")
                nbj = nb_sb[:, j:j + 1]
                # 34:30 ACT:DVE Bresenham split (ACT ~1.11us/drain vs
                # DVE ~1.27us -> equal spans at 34/30)
                if (j * 34) // NPAIR != ((j - 1) * 34) // NPAIR:
                    nc.scalar.add(dst, src, nbj)
                else:
                    nc.vector.tensor_scalar_add(dst, src, nbj)
                if j == out_s + out_n - 1:
                    nc.sync.dma_start(
                        out=o_d[:, out_s:out_s + out_n, :], in_=o_sb)

    nc.compile()
    return nc


def _host_prep(mu_track, Std_inv_track):
    """Block-diagonal pair panels of the residual W[c=2e+pc, j, n=2d+pd] =
    8*R[j+64pd, d, e]*(pc==pd) in fp8e4, and the negated per-partition bias
    nbias[n=2d+p, j] = -8*(R@mu + mu)[d, j+64p] (f32), where R = S - I."""
    import ml_dtypes

    S = np.ascontiguousarray(Std_inv_track, dtype=np.float32)
    mu = np.ascontiguousarray(mu_track, dtype=np.float32)
    R = S - np.eye(D, dtype=np.float32)[None]

    W = np.zeros((2 * D, NPAIR, 2 * D), dtype=np.float32)
    W6 = W.reshape(D, 2, NPAIR, D, 2)                 # [e, pc, j, d, pd]
    R_r = (CSCALE * R).reshape(2, NPAIR, D, D)        # [pk, j, d, e]
    W6[:, 0, :, :, 0] = R_r[0].transpose(2, 0, 1)     # [e, j, d]
    W6[:, 1, :, :, 1] = R_r[1].transpose(2, 0, 1)

    bias_dk = np.einsum("kde,ek->dk", R, mu) + mu     # [d, k], k = 64p + j
    nbias = (-CSCALE) * bias_dk.reshape(D, 2, NPAIR).reshape(2 * D, NPAIR)
    return (W.astype(ml_dtypes.float8_e4m3),
            np.ascontiguousarray(nbias, dtype=np.float32))


def _pack_x(x, n_cores, b_shard):
    """x [n_cores*b_shard, D, K] f32 -> xt [n_cores, 128, NPAIR, b_shard] bf16
    with xt[core, 2e+p, j, b] = x[b, e, j + 64p]."""
    import ml_dtypes

    xb = np.ascontiguousarray(x, dtype=np.float32).astype(ml_dtypes.float8_e4m3)
    xp = xb.reshape(n_cores, b_shard, D, 2, NPAIR)    # [core, b, e, p, j]
    xt = xp.transpose(0, 2, 3, 4, 1)                  # [core, e, p, j, b]
    return np.ascontiguousarray(xt).reshape(n_cores, P, NPAIR, b_shard)


def _unpack_out(oT, x, n_cores, b_shard):
    """oT [n_cores, 128, NPAIR, b_shard] fp8e3 (the scaled correction 8c with
    c[b, d, j+64p] = oT[core, 2d+p, j, b]) -> out = x + c/8 [B, D, K] f32."""
    ov = np.asarray(oT).astype(np.float32)
    ov = ov.reshape(n_cores, D, 2, NPAIR, b_shard)    # [core, d, p, j, b]
    c = ov.transpose(0, 4, 1, 2, 3)                   # [core, b, d, p, j]
    c = np.ascontiguousarray(c).reshape(n_cores * b_shard, D, K)
    return np.asarray(x, dtype=np.float32) + c * (1.0 / CSCALE)


def kernel(x, mu_track, Std_inv_track):
    from concourse.bass_utils import run_bass_kernel_spmd

    xt = _pack_x(x, N_CORES, B_SHARD)
    W, nbias = _host_prep(mu_track, Std_inv_track)

    if "nc" not in _cache:
        _cache["nc"] = _build_nc(B_SHARD)
    nc = _cache["nc"]

    in_maps = []
    for i in range(N_CORES):
        in_maps.append({"xt": xt[i], "w": W, "nbias": nbias})
    res = run_bass_kernel_spmd(nc, in_maps, core_ids=list(range(N_CORES)))
    oT = np.stack([r["out"] for r in res.results], axis=0)
    return _unpack_out(oT, x, N_CORES, B_SHARD)


# revision 16
# speedup vs baseline: 1.2873x; 1.0004x over previous
"""ClusterNorm1d kernel for Trainium2 (Bass/Tile), 8-core data parallel.

out[b,d,k] = sum_e Std_inv[k,d,e] * (x[b,e,k] - mu[e,k])

Strategy (v8, fp8 correction output):
  - Decompose Std_inv = I + R (R ~ 0.01, small). The device computes only
    the *correction* c = 8*[R @ (x - mu) - mu] and the host forms
    out = x + c/8 with its exact f32 copy of x. The identity term never
    moves through the device, so the output stream shrinks to fp8
    (e3m4, 4 mantissa bits; |c| <= ~6 < 15.5 max normal) and the weight
    panels shrink to fp8 (e4m3; values 8*R ~ 0.08..0.4 are all normal).
    Measured end-to-end rel err ~4e-3 against the f32 reference
    (budget 2e-2).
  - Shard batch B=8192 across 8 cores (1024 rows each).
  - Host packs x pre-transposed and pair-interleaved in fp8e4 (x only
    feeds the R-term; the identity term comes from the host's f32 x, so
    fp8 input error contributes ~1e-3):
      xt[c, j, b] = x[b, e, j + 64*p]   with c = 2e + p
    so clusters (j, j+64) share one 128-deep contraction. Weight panels
    are block-diagonal pair panels of 8R in fp8e4:
      W[c=2e+pc, j, n=2d+pd] = 8*R[j+64*pd, d, e] * (pc == pd)
  - Device work per pair j: one stationary-weight matmul (lhsT fp8e4,
    rhs fp8e4 -> psum f32) per 512-row segment, then a PSUM->SBUF drain
    fused with the bias nb[n,j] = -8*(R@mu + mu) (per-partition scalar),
    alternating ACT / DVE, writing fp8e3.
  - DMA: x stream (8.4 MB fp8) + nbias on the SP HWDGE queue; W (1 MB,
    split [8, 56] so the first pairs' weights land ~4us early) + out
    stream (8.4 MB fp8) on the ACT HWDGE queue. ~17.9 MB/core total vs
    ~36 MB for the bf16-everything variant; fabric ceiling ~430 GB/s.
    With DMA off the critical path the kernel is PE-stream-bound:
    64 pairs x 2 x 512-row fp8 matmuls at ~0.74 ns/row (~49 us) plus
    per-pair LDWEIGHTS, ~95% dense once the W head chunk lands.
    (fp8 DoubleRow was tried and is NOT faster on this hardware; it
    runs at ~1 row/cycle in both plain and interleaved layouts.)
"""

import numpy as np

B, D, K = 8192, 64, 128
N_CORES = 8
B_SHARD = B // N_CORES  # 1024
P = 128                 # SBUF partitions
NPAIR = K // 2          # 64 cluster pairs: (j, j+64)
SEG = 512               # matmul moving free-dim per PSUM bank
CSCALE = 8.0            # correction scale: device returns 8*c, host divides

# DMA chunking (pairs per transfer). Packet size per partition row is
# chunk*b_shard*dtype bytes. Head chunks small so compute starts early,
# tail chunks small so the pipeline drains early.
IN_CHUNKS = [4, 8, 16, 16, 16, 2, 1, 1]
OUT_CHUNKS = [16, 16, 16, 8, 4, 2, 1, 1]

_cache = {}


def _bounds(chunks):
    out, s = [], 0
    for c in chunks:
        out.append((s, c))
        s += c
    return out


def _build_nc(b_shard):
    import concourse.tile as tile
    from concourse import bacc, mybir

    f32 = mybir.dt.float32
    bf16 = mybir.dt.bfloat16
    fp8e4 = mybir.dt.float8e4
    fp8e3 = mybir.dt.float8e3
    u8 = mybir.dt.uint8
    nc = bacc.Bacc("TRN2", target_bir_lowering=False)

    xt_d = nc.dram_tensor("xt", [P, NPAIR, b_shard], fp8e4, kind="ExternalInput")
    w_d = nc.dram_tensor("w", [P, NPAIR, P], fp8e4, kind="ExternalInput")
    nb_d = nc.dram_tensor("nbias", [P, NPAIR], f32, kind="ExternalInput")
    o_d = nc.dram_tensor("out", [P, NPAIR, b_shard], fp8e3, kind="ExternalOutput")

    seg = min(SEG, b_shard)
    nseg = b_shard // seg
    in_bounds = _bounds(IN_CHUNKS)
    out_bounds = _bounds(OUT_CHUNKS)

    with tile.TileContext(nc) as tc:
        with (
            tc.tile_pool(name="consts", bufs=1) as consts,
            tc.tile_pool(name="xin", bufs=4) as xin,
            tc.tile_pool(name="oout", bufs=3) as oout,
            tc.tile_pool(name="ps", bufs=4, space="PSUM") as psp,
        ):
            w_sb = consts.tile([P, NPAIR, P], fp8e4)
            nb_sb = consts.tile([P, NPAIR], f32)

            # Engine warm-ups on a zeroed local tile — independent of the
            # const DMAs so they run during the queue spin-up.
            warm_in = consts.tile([P, P], bf16)
            nc.gpsimd.memset(warm_in, 0)
            scratch = consts.tile([P, 2], f32)
            nc.gpsimd.memset(scratch, 0)
            warm_ps = psp.tile([P, 2, seg], f32, tag="ps")
            nc.tensor.matmul(warm_ps[:, 0, 0:P], lhsT=warm_in, rhs=warm_in)
            nc.scalar.copy(out=scratch[:, 0:1], in_=scratch[:, 0:1])
            nc.vector.tensor_copy(out=scratch[:, 1:2], in_=scratch[:, 1:2])

            # HWDGE descriptor generation is ~24ns/descriptor (~3.1us per
            # 128-row DMA entry) and serializes per queue. The SP ring leads
            # with the first x chunks; the fp8 W panel rides the ACT ring
            # ahead of the out stream, split so the first pairs' weights
            # land ~4us earlier than a single 1 MB entry would.
            nc.scalar.dma_start(out=w_sb[:, 0:8, :], in_=w_d[:, 0:8, :])
            nc.scalar.dma_start(out=w_sb[:, 8:NPAIR, :],
                                in_=w_d[:, 8:NPAIR, :])

            in_it = iter(in_bounds)
            out_it = iter(out_bounds)
            xt = o_sb = None
            in_s = in_n = out_s = out_n = 0
            for j in range(NPAIR):
                if xt is None or j >= in_s + in_n:
                    in_s, in_n = next(in_it)
                    xt = xin.tile([P, in_n, b_shard], fp8e4, tag="xt")
                    nc.sync.dma_start(
                        out=xt, in_=xt_d[:, in_s:in_s + in_n, :])
                    if j == 0:
                        nc.sync.dma_start(out=nb_sb, in_=nb_d[:])
                if o_sb is None or j >= out_s + out_n:
                    out_s, out_n = next(out_it)
                    o_sb = oout.tile([P, out_n, b_shard], fp8e3, tag="o")
                # both halves of pair j land in one 2-bank PSUM tile, then
                # drain in a single bias-fused op (alternating ACT/DVE)
                ps = psp.tile([P, nseg, seg], f32, tag="ps")
                for h in range(nseg):
                    nc.tensor.matmul(
                        ps[:, h, :], lhsT=w_sb[:, j, :],
                        rhs=xt[:, j - in_s,
                               h * seg:(h + 1) * seg].bitcast(fp8e4))
                dst = o_sb[:, j - out_s, :]
                src = ps.rearrange("p a b -> p (a b)")
                nbj = nb_sb[:, j:j + 1]
                # 34:30 ACT:DVE Bresenham split (ACT ~1.11us/drain vs
                # DVE ~1.27us -> equal spans at 34/30)
                if (j * 34) // NPAIR != ((j - 1) * 34) // NPAIR:
                    nc.scalar.add(dst, src, nbj)
                else:
                    nc.vector.tensor_scalar_add(dst, src, nbj)
                if j == out_s + out_n - 1:
                    nc.sync.dma_start(
                        out=o_d[:, out_s:out_s + out_n, :], in_=o_sb)

    nc.compile()
    return nc


def _host_prep(mu_track, Std_inv_track):
    """Block-diagonal pair panels of the residual W[c=2e+pc, j, n=2d+pd] =
    8*R[j+64pd, d, e]*(pc==pd) in fp8e4, and the negated per-partition bias
    nbias[n=2d+p, j] = -8*(R@mu + mu)[d, j+64p] (f32), where R = S - I."""
    import ml_dtypes

    S = np.ascontiguousarray(Std_inv_track, dtype=np.float32)
    mu = np.ascontiguousarray(mu_track, dtype=np.float32)
    R = S - np.eye(D, dtype=np.float32)[None]

    W = np.zeros((2 * D, NPAIR, 2 * D), dtype=np.float32)
    W6 = W.reshape(D, 2, NPAIR, D, 2)                 # [e, pc, j, d, pd]
    R_r = (CSCALE * R).reshape(2, NPAIR, D, D)        # [pk, j, d, e]
    W6[:, 0, :, :, 0] = R_r[0].transpose(2, 0, 1)     # [e, j, d]
    W6[:, 1, :, :, 1] = R_r[1].transpose(2, 0, 1)

    bias_dk = np.einsum("kde,ek->dk", R, mu) + mu     # [d, k], k = 64p + j
    nbias = (-CSCALE) * bias_dk.reshape(D, 2, NPAIR).reshape(2 * D, NPAIR)
    return (W.astype(ml_dtypes.float8_e4m3),
            np.ascontiguousarray(nbias, dtype=np.float32))


def _pack_x(x, n_cores, b_shard):
    """x [n_cores*b_shard, D, K] f32 -> xt [n_cores, 128, NPAIR, b_shard] bf16
    with xt[core, 2e+p, j, b] = x[b, e, j + 64p]."""
    import ml_dtypes

    xb = np.ascontiguousarray(x, dtype=np.float32).astype(ml_dtypes.float8_e4m3)
    xp = xb.reshape(n_cores, b_shard, D, 2, NPAIR)    # [core, b, e, p, j]
    xt = xp.transpose(0, 2, 3, 4, 1)                  # [core, e, p, j, b]
    return np.ascontiguousarray(xt).reshape(n_cores, P, NPAIR, b_shard)


def _unpack_out(oT, x, n_cores, b_shard):
    """oT [n_cores, 128, NPAIR, b_shard] fp8e3 (the scaled correction 8c with
    c[b, d, j+64p] = oT[core, 2d+p, j, b]) -> out = x + c/8 [B, D, K] f32."""
    ov = np.asarray(oT).astype(np.float32)
    ov = ov.reshape(n_cores, D, 2, NPAIR, b_shard)    # [core, d, p, j, b]
    c = ov.transpose(0, 4, 1, 2, 3)                   # [core, b, d, p, j]
    c = np.ascontiguousarray(c).reshape(n_cores * b_shard, D, K)
    return np.asarray(x, dtype=np.float32) + c * (1.0 / CSCALE)


def kernel(x, mu_track, Std_inv_track):
    from concourse.bass_utils import run_bass_kernel_spmd

    xt = _pack_x(x, N_CORES, B_SHARD)
    W, nbias = _host_prep(mu_track, Std_inv_track)

    if "nc" not in _cache:
        _cache["nc"] = _build_nc(B_SHARD)
    nc = _cache["nc"]

    in_maps = []
    for i in range(N_CORES):
        in_maps.append({"xt": xt[i], "w": W, "nbias": nbias})
    res = run_bass_kernel_spmd(nc, in_maps, core_ids=list(range(N_CORES)))
    oT = np.stack([r["out"] for r in res.results], axis=0)
    return _unpack_out(oT, x, N_CORES, B_SHARD)
